# revision 11
# baseline (speedup 1.0000x reference)
"""Distributed Trainium2 kernel for the additive-attention alignment predictor.

Math: score[b,t,u] = sum_h w_h * tanh(ep[b,t,h] + dp[b,u,h]);  out = softmax_u(score)
  where ep = enc @ W_enc (bias folded into dp), dp = dec @ W_dec + b_enc + b_dec.
  (b_score is dropped: softmax is shift-invariant.)

Key trick: tanh(z) on z in [-5.8, 5.8] is replaced by the separable expansion
  tanh(z) ~= c1*z + c3*z^3 + sum_f c_f sin(a_f z)
  sin(a(x+y)) = sin(ax)cos(ay) + cos(ax)sin(ay);  z^3 expands into x^i y^j products,
so the whole [T,U,H] contraction becomes TensorEngine matmuls over an expanded
contraction axis, with only O((T+U)*H) ScalarEngine work for the feature planes.

The device Sin table is only accurate for |arg| <= ~3.3, so sin/cos planes are
built via half-angle evaluation (args <= 1.73, cos arg <= 3.3) and a double-angle
ladder (Vector/GpSimd) for the high frequencies {2a, 4a, 2b, 4b}. Sin planes are
stored as sin/2^g (g = ladder generation); the 2^g factors fold into the score
coefficients. Everything is bf16 except the psum accumulators and the softmax:
validated end-to-end rel err ~2.3e-3 in numpy with this exact arithmetic.
Scores are bounded (|score| < 2), so softmax runs without max subtraction.

Sharding: data-parallel over (B, T/2): core c handles batch c//2, t-half c%2.
No cross-core communication.
"""

import math

import numpy as np
import ml_dtypes

import concourse.bass as bass
import concourse.tile as tile
from concourse import bacc, mybir
from concourse.bass_utils import run_bass_kernel_spmd

# Problem shapes (hardcoded per spec)
B, T, U = 4, 800, 150
D, H = 512, 256
NCORES = 8
TPC = T * B // NCORES  # 400 t-rows per core
P = 128
KT = D // P
HT = H // P
EPOFF, DPOFF, EPDPW = 0, TPC, TPC + U
TBLK = [(i * P, min(P, TPC - i * P)) for i in range((TPC + P - 1) // P)]

# Fitted expansion: tanh(z) ~= C1*z + C3*z^3 + sum c_f sin(f z), |z|<=5.8
FD, FA, FB = 1.210016, 0.789398, 1.085813
C1, C3 = 0.456703, -0.009294
CFREQ = {
    "d": 0.225962, "a": -0.011768, "b": 0.051665,
    "2a": 0.018029, "4a": 0.014671, "2b": 0.061241, "4b": 0.003115,
}
GEN = {"d": 1, "a": 1, "b": 1, "2a": 2, "2b": 2, "4a": 3, "4b": 3}
FREQ_ORDER = ["d", "a", "b", "2a", "2b", "4a", "4b"]
# wbt columns: [C1, 3*C3, C3] + per-freq c_f * 2^gen (sin planes stored /2^gen)
WBT_COLS = [C1, 3 * C3, C3] + [CFREQ[f] * (2 ** GEN[f]) for f in FREQ_ORDER]
NWB = len(WBT_COLS)

F32 = mybir.dt.float32
BF16 = mybir.dt.bfloat16
AF = mybir.ActivationFunctionType
ALU = mybir.AluOpType


def _build_graph():
    nc = bacc.Bacc()
    enc_x = nc.declare_dram_parameter("enc_t", [D, TPC], BF16, isOutput=False)
    dec_x = nc.declare_dram_parameter("dec_t", [D, U], BF16, isOutput=False)
    we_x = nc.declare_dram_parameter("w_enc", [D, H], BF16, isOutput=False)
    wd_x = nc.declare_dram_parameter("w_dec", [D, H], BF16, isOutput=False)
    bias_x = nc.declare_dram_parameter("bias2", [P, HT], F32, isOutput=False)
    wbt_x = nc.declare_dram_parameter("wbt", [P, HT, NWB], F32, isOutput=False)
    out_x = nc.declare_dram_parameter("out", [TPC, U], F32, isOutput=True)

    enc_v = enc_x[:].rearrange("(k p) t -> p k t", p=P)
    dec_v = dec_x[:].rearrange("(k p) u -> p k u", p=P)
    we_v = we_x[:].rearrange("(k p) h -> p k h", p=P)
    wd_v = wd_x[:].rearrange("(k p) h -> p k h", p=P)

    with tile.TileContext(nc) as tc:
        with (
            tc.tile_pool(name="const", bufs=1) as const,
            tc.tile_pool(name="tmp", bufs=3) as tmp,
            tc.tile_pool(name="soft", bufs=1) as soft,
            tc.tile_pool(name="ppsum", bufs=1, space="PSUM") as ppsum,
            tc.tile_pool(name="spsum", bufs=1, space="PSUM") as spsum,
        ):
            # ---- inputs to SBUF, k-sliced, on two HWDGE queues (SP + Act)
            enc_sb = const.tile([P, KT, TPC], BF16)
            dec_sb = const.tile([P, KT, U], BF16)
            we_sb = const.tile([P, KT, H], BF16)
            wd_sb = const.tile([P, KT, H], BF16)
            for k in range(KT):
                nc.scalar.dma_start(out=we_sb[:, k, :], in_=we_v[:, k, :])
                nc.sync.dma_start(out=enc_sb[:, k, :], in_=enc_v[:, k, :])
                nc.scalar.dma_start(out=wd_sb[:, k, :], in_=wd_v[:, k, :])
                nc.sync.dma_start(out=dec_sb[:, k, :], in_=dec_v[:, k, :])
            bias_sb = const.tile([P, HT], F32)
            nc.gpsimd.dma_start(out=bias_sb, in_=bias_x[:])
            wbt_sb = const.tile([P, HT, NWB], F32)
            nc.gpsimd.dma_start(out=wbt_sb, in_=wbt_x[:])

            # ---- projections -> epdp (bf16): [p,m,0:TPC]=ep, [..,TPC:]=dp+biases
            epdp = const.tile([P, HT, EPDPW], BF16)
            ps_ep = [ppsum.tile([P, TPC], F32, name=f"ps_ep{m}") for m in range(HT)]
            ps_dp = [ppsum.tile([P, U], F32, name=f"ps_dp{m}") for m in range(HT)]
            for k in range(KT):
                for m in range(HT):
                    nc.tensor.matmul(
                        ps_ep[m],
                        lhsT=we_sb[:, k, m * P : (m + 1) * P],
                        rhs=enc_sb[:, k, :],
                        start=(k == 0),
                        stop=(k == KT - 1),
                    )
                    nc.tensor.matmul(
                        ps_dp[m],
                        lhsT=wd_sb[:, k, m * P : (m + 1) * P],
                        rhs=dec_sb[:, k, :],
                        start=(k == 0),
                        stop=(k == KT - 1),
                    )
            for m in range(HT):
                nc.vector.tensor_copy(epdp[:, m, EPOFF : EPOFF + TPC], ps_ep[m])
                nc.vector.tensor_scalar_add(
                    out=epdp[:, m, DPOFF : DPOFF + U],
                    in0=ps_dp[m],
                    scalar1=bias_sb[:, m : m + 1],
                )

            ones_a = const.tile([P, P], BF16)
            nc.vector.memset(ones_a, 1.0)
            ones_u = const.tile([P, U], BF16)
            nc.vector.memset(ones_u, 1.0)
            halfpi = const.tile([P, 1], F32)
            nc.vector.memset(halfpi, math.pi / 2)

            # squares plane x^2|y^2 and y^3 (dp side, gpsimd)
            sq = const.tile([P, HT, EPDPW], BF16)
            nc.scalar.activation(out=sq, in_=epdp, func=AF.Square, scale=1.0)
            v3 = const.tile([P, HT, U], BF16)
            nc.gpsimd.tensor_tensor(
                out=v3,
                in0=epdp[:, :, DPOFF : DPOFF + U],
                in1=sq[:, :, DPOFF : DPOFF + U],
                op=ALU.mult,
            )

            # trig planes per freq, colocated: trig[nm][:, 0]=sin/2^g, [:, 1]=cos
            trig = {}
            for nm, f in (("d", FD), ("a", FA), ("b", FB)):
                sh = tmp.tile([P, HT, EPDPW], BF16, name="sh")
                nc.scalar.activation(out=sh, in_=epdp, func=AF.Sin, scale=float(f / 2))
                ch = tmp.tile([P, HT, EPDPW], BF16, name="ch")
                nc.scalar.activation(
                    out=ch, in_=epdp, func=AF.Sin, scale=float(f / 2), bias=halfpi[:, :]
                )
                tg = const.tile([P, 2, HT, EPDPW], BF16, name=f"trig_{nm}")
                trig[nm] = tg
                # stored sin/2 = sh*ch
                nc.vector.tensor_tensor(out=tg[:, 0], in0=sh, in1=ch, op=ALU.mult)
                shsq = tmp.tile([P, HT, EPDPW], BF16, name="shsq")
                nc.scalar.activation(out=shsq, in_=sh, func=AF.Square, scale=1.0)
                nc.vector.tensor_scalar(
                    out=tg[:, 1], in0=shsq, scalar1=-2.0, scalar2=1.0,
                    op0=ALU.mult, op1=ALU.add,
                )
            for src, dst in (("a", "2a"), ("2a", "4a"), ("b", "2b"), ("2b", "4b")):
                ts_, td = trig[src], const.tile([P, 2, HT, EPDPW], BF16, name=f"trig_{dst}")
                trig[dst] = td
                # stored sin_dst/2^(g+1) = (sin_src/2^g) * cos_src
                nc.vector.tensor_tensor(out=td[:, 0], in0=ts_[:, 0], in1=ts_[:, 1], op=ALU.mult)
                ssq = tmp.tile([P, HT, EPDPW], BF16, name="shsq")
                nc.vector.tensor_tensor(out=ssq, in0=ts_[:, 0], in1=ts_[:, 0], op=ALU.mult)
                # cos_dst = 1 - 2*sin_src^2 = 1 - 2*4^g * stored^2
                nc.vector.tensor_scalar(
                    out=td[:, 1], in0=ssq, scalar1=float(-2.0 * 4 ** GEN[src]),
                    scalar2=1.0, op0=ALU.mult, op1=ALU.add,
                )

            # ---- B-side planes scaled by (coef * w_h), on gpsimd (DVE is busier)
            def make_b(src, col, name, eng):
                bt = const.tile([P, HT, U], BF16, name=name)
                for m in range(HT):
                    eng.tensor_scalar_mul(
                        out=bt[:, m, :],
                        in0=(ones_u if src is None else src[:, m, DPOFF : DPOFF + U]),
                        scalar1=wbt_sb[:, m, col : col + 1],
                    )
                return bt

            b_one = make_b(None, 0, "b_one", nc.gpsimd)
            b_y1 = make_b(epdp, 0, "b_y1", nc.gpsimd)
            b_y3c = make_b(epdp, 1, "b_y3c", nc.gpsimd)
            b_sq = make_b(sq, 1, "b_sq", nc.gpsimd)
            # v3 is [P, HT, U] already (no EPDPW offset)
            b_cu = const.tile([P, HT, U], BF16, name="b_cu")
            for m in range(HT):
                nc.gpsimd.tensor_scalar_mul(
                    out=b_cu[:, m, :], in0=v3[:, m, :], scalar1=wbt_sb[:, m, 2:3]
                )
            # freq B planes: scale sin&cos dp-parts in one op per (freq, htile)
            b_trig = {}
            for i, nm in enumerate(FREQ_ORDER):
                bt = const.tile([P, 2, HT, U], BF16, name=f"b_trig_{nm}")
                b_trig[nm] = bt
                for m in range(HT):
                    nc.gpsimd.tensor_scalar_mul(
                        out=bt[:, :, m, :],
                        in0=trig[nm][:, :, m, DPOFF : DPOFF + U],
                        scalar1=wbt_sb[:, m, 3 + i : 4 + i],
                    )

            # ---- matmul pairs: (A_tile_or_ones, A_slice_fn, B_ap_fn)
            pairs = []
            pairs.append((lambda m, s: epdp[:, m, s], lambda m: b_one[:, m, :]))
            pairs.append((lambda m, s: ones_a[:, : s.stop - s.start], lambda m: b_y1[:, m, :]))
            pairs.append((lambda m, s: sq[:, m, s], lambda m: b_y3c[:, m, :]))
            pairs.append((lambda m, s: epdp[:, m, s], lambda m: b_sq[:, m, :]))
            pairs.append((lambda m, s: ones_a[:, : s.stop - s.start], lambda m: b_cu[:, m, :]))
            for nm in FREQ_ORDER:
                tg, bt = trig[nm], b_trig[nm]
                pairs.append((lambda m, s, tg=tg: tg[:, 0, m, s], lambda m, bt=bt: bt[:, 1, m, :]))
                pairs.append((lambda m, s, tg=tg: tg[:, 1, m, s], lambda m, bt=bt: bt[:, 0, m, :]))

            # single psum tile, one bank per t-block
            sp = spsum.tile([P, len(TBLK), 512], F32)
            n_mm = 2 * len(pairs)
            for tb, (t0, pn) in enumerate(TBLK):
                i = 0
                sl = slice(EPOFF + t0, EPOFF + t0 + pn)
                for a_fn, b_fn in pairs:
                    for m in range(HT):
                        nc.tensor.matmul(
                            sp[:pn, tb, 0:U],
                            lhsT=a_fn(m, sl),
                            rhs=b_fn(m),
                            start=(i == 0),
                            stop=(i == n_mm - 1),
                        )
                        i += 1

            # ---- softmax over u (scores bounded, no max subtraction), all blocks
            expt = soft.tile([P, len(TBLK), U], F32)
            nc.scalar.activation(out=expt, in_=sp[:, :, 0:U], func=AF.Exp, scale=1.0)
            ssum = soft.tile([P, len(TBLK)], F32)
            nc.vector.tensor_reduce(
                out=ssum, in_=expt, axis=mybir.AxisListType.X, op=ALU.add
            )
            nc.vector.reciprocal(out=ssum, in_=ssum)
            outt = soft.tile([P, len(TBLK), U], F32)
            for tb, (t0, pn) in enumerate(TBLK):
                nc.vector.tensor_scalar_mul(
                    out=outt[:pn, tb, :], in0=expt[:pn, tb, :],
                    scalar1=ssum[:pn, tb : tb + 1],
                )
                nc.sync.dma_start(out=out_x[t0 : t0 + pn, :], in_=outt[:pn, tb, :])

    nc.finalize()
    return nc


_NC_CACHE = None


def kernel(**inputs: np.ndarray) -> np.ndarray:
    global _NC_CACHE
    bf = ml_dtypes.bfloat16
    enc = np.asarray(inputs["encoder_out"], dtype=np.float32)
    dec = np.asarray(inputs["decoder_out"], dtype=np.float32)
    w_enc = np.ascontiguousarray(np.asarray(inputs["W_enc"], np.float32).astype(bf))
    b_enc = np.asarray(inputs["b_enc"], dtype=np.float32)
    w_dec = np.ascontiguousarray(np.asarray(inputs["W_dec"], np.float32).astype(bf))
    b_dec = np.asarray(inputs["b_dec"], dtype=np.float32)
    w_score = np.asarray(inputs["w_score"], dtype=np.float32)
    # b_score dropped: softmax(x + c) == softmax(x)

    bias2 = np.ascontiguousarray((b_enc + b_dec).reshape(HT, P).T)
    wbt = np.empty((P, HT, NWB), dtype=np.float32)
    for m in range(HT):
        wseg = w_score[m * P : (m + 1) * P]
        for j, c in enumerate(WBT_COLS):
            wbt[:, m, j] = np.float32(c) * wseg
    wbt = np.ascontiguousarray(wbt)

    in_maps = []
    for c in range(NCORES):
        b = c // (NCORES // B)
        t0 = (c % (NCORES // B)) * TPC
        in_maps.append(
            {
                "enc_t": np.ascontiguousarray(enc[b, t0 : t0 + TPC, :].T.astype(bf)),
                "dec_t": np.ascontiguousarray(dec[b].T.astype(bf)),
                "w_enc": w_enc,
                "w_dec": w_dec,
                "bias2": bias2,
                "wbt": wbt,
            }
        )

    if _NC_CACHE is None:
        _NC_CACHE = _build_graph()
    res = run_bass_kernel_spmd(_NC_CACHE, in_maps, core_ids=list(range(NCORES)))

    out = np.empty((B, T, U), dtype=np.float32)
    for c in range(NCORES):
        b = c // (NCORES // B)
        t0 = (c % (NCORES // B)) * TPC
        out[b, t0 : t0 + TPC, :] = res.results[c]["out"]
    return out


# revision 12
# speedup vs baseline: 2.1275x; 2.1275x over previous
"""Distributed Trainium2 kernel for the additive-attention alignment predictor.

Math: score[b,t,u] = sum_h w_h * tanh(ep[b,t,h] + dp[b,u,h]);  out = softmax_u(score)
  where ep = enc @ W_enc (bias folded into dp), dp = dec @ W_dec + b_enc + b_dec.
  (b_score is dropped: softmax is shift-invariant.)

Key trick: tanh(z) on z in [-5.8, 5.8] is replaced by the separable expansion
  tanh(z) ~= c1*z + c3*z^3 + sum_f c_f sin(a_f z)
  sin(a(x+y)) = sin(ax)cos(ay) + cos(ax)sin(ay);  z^3 expands into x^i y^j products,
so the whole [T,U,H] contraction becomes TensorEngine matmuls over an expanded
contraction axis, with only O((T+U)*H) ScalarEngine work for the feature planes.

The device Sin table is only accurate for |arg| <= ~3.3, so sin/cos planes are
built via half-angle evaluation (args <= 1.73, cos arg <= 3.3) and a double-angle
ladder (Vector/GpSimd) for the high frequencies {2a, 4a, 2b, 4b}. Sin planes are
stored as sin/2^g (g = ladder generation); the 2^g factors fold into the score
coefficients. Everything is bf16 except the psum accumulators and the softmax:
validated end-to-end rel err ~2.3e-3 in numpy with this exact arithmetic.
Scores are bounded (|score| < 2), so softmax runs without max subtraction.

Sharding: data-parallel over (B, T/2): core c handles batch c//2, t-half c%2.
No cross-core communication.
"""

import math

import numpy as np
import ml_dtypes

import concourse.bass as bass
import concourse.tile as tile
from concourse import bacc, mybir
from concourse.bass_utils import run_bass_kernel_spmd

# Problem shapes (hardcoded per spec)
B, T, U = 4, 800, 150
D, H = 512, 256
NCORES = 8
TPC = T * B // NCORES  # 400 t-rows per core
P = 128
KT = D // P
HT = H // P
EPOFF, DPOFF, EPDPW = 0, TPC, TPC + U
TBLK = [(i * P, min(P, TPC - i * P)) for i in range((TPC + P - 1) // P)]

# Fitted expansion: tanh(z) ~= C1*z + C3*z^3 + sum c_f sin(f z), |z|<=5.8
FD, FA, FB = 1.210016, 0.789398, 1.085813
C1, C3 = 0.456703, -0.009294
CFREQ = {
    "d": 0.225962, "a": -0.011768, "b": 0.051665,
    "2a": 0.018029, "4a": 0.014671, "2b": 0.061241, "4b": 0.003115,
}
GEN = {"d": 1, "a": 1, "b": 1, "2a": 2, "2b": 2, "4a": 3, "4b": 3}
FREQ_ORDER = ["d", "a", "b", "2a", "2b", "4a", "4b"]
# wbt columns: [C1, 3*C3, C3] + per-freq c_f * 2^gen (sin planes stored /2^gen)
WBT_COLS = [C1, 3 * C3, C3] + [CFREQ[f] * (2 ** GEN[f]) for f in FREQ_ORDER]
NWB = len(WBT_COLS)

F32 = mybir.dt.float32
BF16 = mybir.dt.bfloat16
AF = mybir.ActivationFunctionType
ALU = mybir.AluOpType


def _build_graph():
    nc = bacc.Bacc()
    enc_x = nc.declare_dram_parameter("enc_t", [D, TPC], BF16, isOutput=False)
    dec_x = nc.declare_dram_parameter("dec_t", [D, U], BF16, isOutput=False)
    we_x = nc.declare_dram_parameter("w_enc", [D, H], BF16, isOutput=False)
    wd_x = nc.declare_dram_parameter("w_dec", [D, H], BF16, isOutput=False)
    bias_x = nc.declare_dram_parameter("bias2", [P, HT], F32, isOutput=False)
    wbt_x = nc.declare_dram_parameter("wbt", [P, HT, NWB], F32, isOutput=False)
    out_x = nc.declare_dram_parameter("out", [TPC, U], F32, isOutput=True)

    enc_v = enc_x[:].rearrange("(k p) t -> p k t", p=P)
    dec_v = dec_x[:].rearrange("(k p) u -> p k u", p=P)
    we_v = we_x[:].rearrange("(k p) h -> p k h", p=P)
    wd_v = wd_x[:].rearrange("(k p) h -> p k h", p=P)

    with tile.TileContext(nc) as tc:
        with (
            tc.tile_pool(name="const", bufs=1) as const,
            tc.tile_pool(name="tmp", bufs=3) as tmp,
            tc.tile_pool(name="soft", bufs=1) as soft,
            tc.tile_pool(name="ppsum", bufs=1, space="PSUM") as ppsum,
            tc.tile_pool(name="spsum", bufs=1, space="PSUM") as spsum,
        ):
            # ---- inputs to SBUF, k-sliced, on two HWDGE queues (SP + Act)
            enc_sb = const.tile([P, KT, TPC], BF16)
            dec_sb = const.tile([P, KT, U], BF16)
            we_sb = const.tile([P, KT, H], BF16)
            wd_sb = const.tile([P, KT, H], BF16)
            for k in range(KT):
                nc.scalar.dma_start(out=we_sb[:, k, :], in_=we_v[:, k, :])
                nc.sync.dma_start(out=enc_sb[:, k, :], in_=enc_v[:, k, :])
                nc.scalar.dma_start(out=wd_sb[:, k, :], in_=wd_v[:, k, :])
                nc.sync.dma_start(out=dec_sb[:, k, :], in_=dec_v[:, k, :])
            bias_sb = const.tile([P, HT], F32)
            nc.gpsimd.dma_start(out=bias_sb, in_=bias_x[:])
            wbt_sb = const.tile([P, HT, NWB], F32)
            nc.gpsimd.dma_start(out=wbt_sb, in_=wbt_x[:])

            # ---- projections -> epdp (bf16): [p,m,0:TPC]=ep, [..,TPC:]=dp+biases
            epdp = const.tile([P, HT, EPDPW], BF16)
            ps_ep = [ppsum.tile([P, TPC], F32, name=f"ps_ep{m}") for m in range(HT)]
            ps_dp = [ppsum.tile([P, U], F32, name=f"ps_dp{m}") for m in range(HT)]
            for k in range(KT):
                for m in range(HT):
                    nc.tensor.matmul(
                        ps_ep[m],
                        lhsT=we_sb[:, k, m * P : (m + 1) * P],
                        rhs=enc_sb[:, k, :],
                        start=(k == 0),
                        stop=(k == KT - 1),
                    )
                    nc.tensor.matmul(
                        ps_dp[m],
                        lhsT=wd_sb[:, k, m * P : (m + 1) * P],
                        rhs=dec_sb[:, k, :],
                        start=(k == 0),
                        stop=(k == KT - 1),
                    )
            for m in range(HT):
                nc.vector.tensor_copy(epdp[:, m, EPOFF : EPOFF + TPC], ps_ep[m])
                nc.vector.tensor_scalar_add(
                    out=epdp[:, m, DPOFF : DPOFF + U],
                    in0=ps_dp[m],
                    scalar1=bias_sb[:, m : m + 1],
                )

            ones_a = const.tile([P, P], BF16)
            nc.vector.memset(ones_a, 1.0)
            ones_u = const.tile([P, U], BF16)
            nc.vector.memset(ones_u, 1.0)
            halfpi = const.tile([P, 1], F32)
            nc.vector.memset(halfpi, math.pi / 2)

            # squares plane x^2|y^2 and y^3 (dp side)
            sq = const.tile([P, HT, EPDPW], BF16)
            nc.scalar.activation(out=sq, in_=epdp, func=AF.Square, scale=1.0)
            v3 = const.tile([P, HT, U], BF16)
            nc.vector.tensor_tensor(
                out=v3,
                in0=epdp[:, :, DPOFF : DPOFF + U],
                in1=sq[:, :, DPOFF : DPOFF + U],
                op=ALU.mult,
            )

            # trig planes per freq (plain tiles: slices of >2-d tiles lose DVE
            # perf modes). sin planes stored as sin/2^gen.
            sin_p, cos_p = {}, {}
            for nm, f in (("d", FD), ("a", FA), ("b", FB)):
                sh = tmp.tile([P, HT, EPDPW], BF16, name="sh")
                nc.scalar.activation(out=sh, in_=epdp, func=AF.Sin, scale=float(f / 2))
                ch = tmp.tile([P, HT, EPDPW], BF16, name="ch")
                nc.scalar.activation(
                    out=ch, in_=epdp, func=AF.Sin, scale=float(f / 2), bias=halfpi[:, :]
                )
                sin_p[nm] = const.tile([P, HT, EPDPW], BF16, name=f"sin_{nm}")
                nc.vector.tensor_tensor(out=sin_p[nm], in0=sh, in1=ch, op=ALU.mult)
                shsq = tmp.tile([P, HT, EPDPW], BF16, name="shsq")
                nc.scalar.activation(out=shsq, in_=sh, func=AF.Square, scale=1.0)
                cos_p[nm] = const.tile([P, HT, EPDPW], BF16, name=f"cos_{nm}")
                nc.vector.tensor_scalar(
                    out=cos_p[nm], in0=shsq, scalar1=-2.0, scalar2=1.0,
                    op0=ALU.mult, op1=ALU.add,
                )
            for i, (src, dst) in enumerate((("a", "2a"), ("2a", "4a"), ("b", "2b"), ("2b", "4b"))):
                sin_p[dst] = const.tile([P, HT, EPDPW], BF16, name=f"sin_{dst}")
                nc.vector.tensor_tensor(
                    out=sin_p[dst], in0=sin_p[src], in1=cos_p[src], op=ALU.mult
                )
                ssq = tmp.tile([P, HT, EPDPW], BF16, name="shsq")
                if i % 2 == 0:  # split square work between ACT and DVE
                    nc.scalar.activation(out=ssq, in_=sin_p[src], func=AF.Square, scale=1.0)
                else:
                    nc.vector.tensor_tensor(out=ssq, in0=sin_p[src], in1=sin_p[src], op=ALU.mult)
                cos_p[dst] = const.tile([P, HT, EPDPW], BF16, name=f"cos_{dst}")
                # cos_dst = 1 - 2*sin_src^2 = 1 - 2*4^g * stored^2
                nc.vector.tensor_scalar(
                    out=cos_p[dst], in0=ssq, scalar1=float(-2.0 * 4 ** GEN[src]),
                    scalar2=1.0, op0=ALU.mult, op1=ALU.add,
                )

            # ---- B-side planes scaled by (coef * w_h) on DVE
            def make_b(src, col, name):
                bt = const.tile([P, HT, U], BF16, name=name)
                for m in range(HT):
                    nc.vector.tensor_scalar_mul(
                        out=bt[:, m, :],
                        in0=(ones_u if src is None else src[:, m, DPOFF : DPOFF + U]),
                        scalar1=wbt_sb[:, m, col : col + 1],
                    )
                return bt

            b_one = make_b(None, 0, "b_one")
            b_y1 = make_b(epdp, 0, "b_y1")
            b_y3c = make_b(epdp, 1, "b_y3c")
            b_sq = make_b(sq, 1, "b_sq")
            b_cu = const.tile([P, HT, U], BF16, name="b_cu")
            for m in range(HT):
                nc.vector.tensor_scalar_mul(
                    out=b_cu[:, m, :], in0=v3[:, m, :], scalar1=wbt_sb[:, m, 2:3]
                )
            b_sin, b_cos = {}, {}
            for i, nm in enumerate(FREQ_ORDER):
                b_sin[nm] = make_b(sin_p[nm], 3 + i, f"b_sin_{nm}")
                b_cos[nm] = make_b(cos_p[nm], 3 + i, f"b_cos_{nm}")

            # ---- matmul pairs: (A_slice_fn, B_ap_fn)
            pairs = []
            pairs.append((lambda m, s: epdp[:, m, s], lambda m: b_one[:, m, :]))
            pairs.append((lambda m, s: ones_a[:, : s.stop - s.start], lambda m: b_y1[:, m, :]))
            pairs.append((lambda m, s: sq[:, m, s], lambda m: b_y3c[:, m, :]))
            pairs.append((lambda m, s: epdp[:, m, s], lambda m: b_sq[:, m, :]))
            pairs.append((lambda m, s: ones_a[:, : s.stop - s.start], lambda m: b_cu[:, m, :]))
            for nm in FREQ_ORDER:
                sp_t, cp_t = sin_p[nm], cos_p[nm]
                bs_t, bc_t = b_sin[nm], b_cos[nm]
                pairs.append((lambda m, s, t=sp_t: t[:, m, s], lambda m, t=bc_t: t[:, m, :]))
                pairs.append((lambda m, s, t=cp_t: t[:, m, s], lambda m, t=bs_t: t[:, m, :]))

            # single psum tile, one bank per t-block
            sp = spsum.tile([P, len(TBLK), 512], F32)
            n_mm = 2 * len(pairs)
            for tb, (t0, pn) in enumerate(TBLK):
                i = 0
                sl = slice(EPOFF + t0, EPOFF + t0 + pn)
                for a_fn, b_fn in pairs:
                    for m in range(HT):
                        nc.tensor.matmul(
                            sp[:pn, tb, 0:U],
                            lhsT=a_fn(m, sl),
                            rhs=b_fn(m),
                            start=(i == 0),
                            stop=(i == n_mm - 1),
                        )
                        i += 1

            # ---- softmax over u (scores bounded, no max subtraction), all blocks
            expt = soft.tile([P, len(TBLK), U], F32)
            nc.scalar.activation(out=expt, in_=sp[:, :, 0:U], func=AF.Exp, scale=1.0)
            ssum = soft.tile([P, len(TBLK)], F32)
            nc.vector.tensor_reduce(
                out=ssum, in_=expt, axis=mybir.AxisListType.X, op=ALU.add
            )
            nc.vector.reciprocal(out=ssum, in_=ssum)
            outt = soft.tile([P, len(TBLK), U], F32)
            for tb, (t0, pn) in enumerate(TBLK):
                nc.vector.tensor_scalar_mul(
                    out=outt[:pn, tb, :], in0=expt[:pn, tb, :],
                    scalar1=ssum[:pn, tb : tb + 1],
                )
                nc.sync.dma_start(out=out_x[t0 : t0 + pn, :], in_=outt[:pn, tb, :])

    nc.finalize()
    return nc


_NC_CACHE = None


def kernel(**inputs: np.ndarray) -> np.ndarray:
    global _NC_CACHE
    bf = ml_dtypes.bfloat16
    enc = np.asarray(inputs["encoder_out"], dtype=np.float32)
    dec = np.asarray(inputs["decoder_out"], dtype=np.float32)
    w_enc = np.ascontiguousarray(np.asarray(inputs["W_enc"], np.float32).astype(bf))
    b_enc = np.asarray(inputs["b_enc"], dtype=np.float32)
    w_dec = np.ascontiguousarray(np.asarray(inputs["W_dec"], np.float32).astype(bf))
    b_dec = np.asarray(inputs["b_dec"], dtype=np.float32)
    w_score = np.asarray(inputs["w_score"], dtype=np.float32)
    # b_score dropped: softmax(x + c) == softmax(x)

    bias2 = np.ascontiguousarray((b_enc + b_dec).reshape(HT, P).T)
    wbt = np.empty((P, HT, NWB), dtype=np.float32)
    for m in range(HT):
        wseg = w_score[m * P : (m + 1) * P]
        for j, c in enumerate(WBT_COLS):
            wbt[:, m, j] = np.float32(c) * wseg
    wbt = np.ascontiguousarray(wbt)

    in_maps = []
    for c in range(NCORES):
        b = c // (NCORES // B)
        t0 = (c % (NCORES // B)) * TPC
        in_maps.append(
            {
                "enc_t": np.ascontiguousarray(enc[b, t0 : t0 + TPC, :].T.astype(bf)),
                "dec_t": np.ascontiguousarray(dec[b].T.astype(bf)),
                "w_enc": w_enc,
                "w_dec": w_dec,
                "bias2": bias2,
                "wbt": wbt,
            }
        )

    if _NC_CACHE is None:
        _NC_CACHE = _build_graph()
    res = run_bass_kernel_spmd(_NC_CACHE, in_maps, core_ids=list(range(NCORES)))

    out = np.empty((B, T, U), dtype=np.float32)
    for c in range(NCORES):
        b = c // (NCORES // B)
        t0 = (c % (NCORES // B)) * TPC
        out[b, t0 : t0 + TPC, :] = res.results[c]["out"]
    return out
